# revision 6
# baseline (speedup 1.0000x reference)
"""Stereo correlation cost volume kernel for Trainium2 (8 NeuronCores).

  out[b, d, h, w] = mean_c( L[b,c,h,w] * R[b,c,h,w-d] )  for w >= d, else 0
  B=8, C=64, H=128, W=256, D=64.

Sharding: data-parallel over batch; core b handles batch b.

Per-core algorithm (w-major blocking, h-interleaved scratch):
  The host ships, per (c, h) row: [L/8 (256) | R reversed /8 (256)], f16;
  64 zero columns per row are memset on-device.  For each h and each
  64-wide w-block j, one matmul
    GT_j[t, m] = sum_c L[c, 64j+t] * R[c, 64j+63-m]        (PSUM fp32)
  directly produces the *transposed* Gram restricted to the 128-wide
  u-window the band needs (u = 64j+63-m; the reversed-R layout makes the
  window a contiguous rhs slice, and for j=0 the u<0 columns read the
  zero pad, so the w<d triangle comes out exactly 0).  Two adjacent
  blocks share one PSUM tile (partitions 0-63 / 64-127 via matmul
  tile_position), so the PSUM->SBUF cast copies run at full 128-lane
  width.  The casts write an h-interleaved (E=16) scratch image
    scr_g[par*262144 + t*4096 + jj*2048 + m*16 + hh]      (j = 2*jj+par)
  so one DMA per (group, par) writes 0.5MB with 8KB-contiguous runs,
  and the band element out[h=16g+hh, w=64j+t, d] = GT_j[t, 63-t+d]
  sits at  par*262144 + jj*2048 + 1008 + t*4080 + 16d + hh  — affine
  with a 2KB-contiguous (d, hh) inner run.  A DRAM->DRAM DMA per
  (group, par, jj) scatters the band straight into the f16 output laid
  out as [g, w, d, hh], all at DMA line rate.  DMA work spreads over
  all three trigger queues (sync/SP + scalar/ACT HWDGE, gpsimd SWDGE);
  the whole input stays SBUF-resident via 4 x 2.1MB loads.  The host
  reshapes [g, w, d, hh] -> [d, h, w] and casts to fp32.
"""

import os
import sys
import types

import numpy as np

sys.path.insert(0, "/opt/trn_rl_repo")


def _register_axon_hooks():
    # Optional: lets run_bass_kernel_spmd(trace=True) collect NTFF
    # profiles under axon when the image's antenv lacks axon_hooks.
    try:
        import antenv.axon_hooks  # noqa: F401
        return
    except ImportError:
        pass
    try:
        import antenv
        from trn_agent_boot.trn_boot import _ntff_profile_via_ctypes

        m = types.ModuleType("antenv.axon_hooks")
        _hook = _ntff_profile_via_ctypes("/opt/axon/libaxon_pjrt.so")
        m.get_axon_ntff_profile_hook = lambda: _hook
        m.set_axon_ntff_profile_hook = lambda h: None
        sys.modules["antenv.axon_hooks"] = m
        antenv.axon_hooks = m
    except Exception:
        pass


_register_axon_hooks()

import ml_dtypes  # noqa: E402, F401

import concourse.bass as bass  # noqa: E402, F401
import concourse.bacc as bacc  # noqa: E402
import concourse.mybir as mybir  # noqa: E402
from concourse.bass import AP  # noqa: E402
from concourse.bass_utils import run_bass_kernel_spmd  # noqa: E402
from concourse.tile import TileContext  # noqa: E402

B, C, H, W = 8, 64, 128, 256
D = 64
NW = 576  # per-(c,h) SBUF input row: [L 256 | Rrev 256 | Z 64]
NHOST = 512  # host only ships [L | Rrev]
NH = 16  # h rows per group (= interleave factor E)
NG = H // NH  # 8 groups
LBH = 32  # h rows per input-load block
GSLOT = 2 * 262144  # scratch elems per group
F32 = mybir.dt.float32
F16 = mybir.dt.float16

_CACHE = {}


def build():
    nc = bacc.Bacc()
    lr_dram = nc.dram_tensor("lr", [C, H, NHOST], F16, kind="ExternalInput")
    # out[g, w, d, hh]
    out_dram = nc.dram_tensor("out", [NG, W, D, NH], F16, kind="ExternalOutput")
    scr = [
        nc.dram_tensor(f"scratch{k}", [GSLOT], F16, kind="Internal")
        for k in range(NG)
    ]

    with TileContext(nc) as tc:
        with (
            tc.tile_pool(name="inp", bufs=H // LBH) as pin,
            tc.tile_pool(name="gband", bufs=3) as pg,
            tc.tile_pool(name="ps", bufs=6, space="PSUM") as pps,
        ):
            # whole input resident: 4 loads of 2.1MB on the gpsimd queue,
            # Z columns memset once per tile
            lrt = []
            for k in range(H // LBH):
                t = pin.tile([C, LBH * NW], F16, tag="lr8")
                tv = t.rearrange("p (h x) -> p h x", h=LBH)
                nc.gpsimd.memset(tv[:, :, NHOST:NW], 0.0)
                nc.gpsimd.dma_start(
                    out=tv[:, :, 0:NHOST],
                    in_=lr_dram[:, k * LBH : (k + 1) * LBH, :],
                )
                lrt.append(tv)

            ncopy = 0
            nband = 0
            for g in range(NG):
                h0 = g * NH

                # stage 2: matmul pairs + strided PSUM->SBUF casts
                # g8[p=(par,t), hh + 16*(m + 128*jj)]
                g8 = pg.tile([128, 2 * 128 * NH], F16, tag="g8")
                g8v = g8.rearrange("p (jj m hh) -> p jj m hh", jj=2, m=128)
                for hh in range(NH):
                    lrv = lrt[(h0 + hh) // LBH]
                    lhh = (h0 + hh) % LBH
                    for jj in range(2):
                        pt = pps.tile([128, 128], F32, tag="pt")
                        for par in range(2):
                            j = 2 * jj + par
                            nc.tensor.matmul(
                                pt[par * 64 : par * 64 + 64, :],
                                lhsT=lrv[:, lhh, 64 * j : 64 * j + 64],
                                rhs=lrv[:, lhh, 448 - 64 * j : 576 - 64 * j],
                            )
                        if ncopy % 8 < 5:
                            nc.vector.tensor_copy(g8v[:, jj, :, hh], pt)
                        else:
                            nc.scalar.copy(g8v[:, jj, :, hh], pt)
                        ncopy += 1

                # stage 3: scratch write, one 0.5MB DMA per (group, par),
                # 8KB contiguous runs
                for par in range(2):
                    eng = nc.sync if par == 0 else nc.scalar
                    eng.dma_start(
                        out=AP(
                            scr[g],
                            par * 262144,
                            [[4096, 64], [1, 4096]],
                        ),
                        in_=g8[par * 64 : par * 64 + 64, :],
                    )

                # stage 4: band extraction, DRAM->DRAM, 2KB runs:
                # out[g, w=64j+t, d, hh] = scr_g[par*262144 + jj*2048
                #                                + 1008 + t*4080 + 16d + hh]
                for par in range(2):
                    for jj in range(2):
                        eng = (nc.sync, nc.scalar, nc.gpsimd)[nband % 3]
                        nband += 1
                        eng.dma_start(
                            out=AP(
                                out_dram,
                                g * (W * D * NH)
                                + (128 * jj + 64 * par) * (D * NH),
                                [[D * NH, 64], [1, D * NH]],
                            ),
                            in_=AP(
                                scr[g],
                                par * 262144 + jj * 2048 + 1008,
                                [[4080, 64], [1, D * NH]],
                            ),
                        )
    nc.finalize()
    return nc


def kernel(left_feature, right_feature, max_disp):
    assert int(max_disp) == D
    left = np.asarray(left_feature, dtype=np.float32)
    right = np.asarray(right_feature, dtype=np.float32)
    assert left.shape == (B, C, H, W) and right.shape == (B, C, H, W)

    if "nc" not in _CACHE:
        _CACHE["nc"] = build()
    nc = _CACHE["nc"]

    in_maps = []
    for b in range(B):
        lr = np.empty((C, H, NHOST), dtype=np.float16)
        lr[:, :, 0:W] = left[b] / 8
        lr[:, :, W : 2 * W] = right[b, :, :, ::-1] / 8
        in_maps.append({"lr": lr})
    res = run_bass_kernel_spmd(nc, in_maps, list(range(B)))
    _CACHE["last_results"] = res
    out = np.stack(
        [
            # [g, w, d, hh] -> [d, (g, hh), w]
            res.results[b]["out"].transpose(2, 0, 3, 1).reshape(D, H, W)
            for b in range(B)
        ],
        axis=0,
    )
    return out.astype(np.float32)


# revision 8
# speedup vs baseline: 1.2972x; 1.2972x over previous
"""Stereo correlation cost volume kernel for Trainium2 (8 NeuronCores).

  out[b, d, h, w] = mean_c( L[b,c,h,w] * R[b,c,h,w-d] )  for w >= d, else 0
  B=8, C=64, H=128, W=256, D=64.

Sharding: data-parallel over batch; core b handles batch b.

Per-core algorithm (w-major blocking, no PE transposes):
  The host ships, per (c, h) row: [L/8 (256) | R reversed /8 (256)], f16;
  64 zero columns per row are memset on-device.  For each h and each
  64-wide w-block j, one matmul
    GT_j[t, m] = sum_c L[c, 64j+t] * R[c, 64j+63-m]        (PSUM fp32)
  directly produces the *transposed* Gram restricted to the 128-wide
  u-window the band needs (u = 64j+63-m; the reversed-R layout makes the
  window a contiguous rhs slice, and for j=0 the u<0 columns read the
  zero pad, so the w<d triangle comes out exactly 0).  Two adjacent
  blocks share one PSUM tile (partitions 0-63 / 64-127 via matmul
  tile_position), so the PSUM->SBUF cast copies run at full 128-lane
  width.  Tiles go to a DRAM scratch with 512-byte-contiguous runs:
    scr[h, par, t, jj, m] at h*32768 + par*16384 + t*256 + jj*128 + m
  (j = 2*jj + par).  The band element out[h, w=64j+t, d] = GT_j[t, 63-t+d]
  then sits at an affine address (... + t*255 + d), so a single
  DRAM->DRAM DMA per (32-h block, par, jj) scatters the band straight
  into the f16 output laid out as [h, w, d].  DMAs are batched large
  (whole input resident in SBUF; 0.5MB scratch writes) and spread over
  all three trigger queues (sync/SP + scalar/ACT HWDGE, gpsimd SWDGE).
  The host transposes to [d, h, w] and casts to fp32.
"""

import os
import sys
import types

import numpy as np

sys.path.insert(0, "/opt/trn_rl_repo")


def _register_axon_hooks():
    # Optional: lets run_bass_kernel_spmd(trace=True) collect NTFF
    # profiles under axon when the image's antenv lacks axon_hooks.
    try:
        import antenv.axon_hooks  # noqa: F401
        return
    except ImportError:
        pass
    try:
        import antenv
        from trn_agent_boot.trn_boot import _ntff_profile_via_ctypes

        m = types.ModuleType("antenv.axon_hooks")
        _hook = _ntff_profile_via_ctypes("/opt/axon/libaxon_pjrt.so")
        m.get_axon_ntff_profile_hook = lambda: _hook
        m.set_axon_ntff_profile_hook = lambda h: None
        sys.modules["antenv.axon_hooks"] = m
        antenv.axon_hooks = m
    except Exception:
        pass


_register_axon_hooks()

import ml_dtypes  # noqa: E402, F401

import concourse.bass as bass  # noqa: E402, F401
import concourse.bacc as bacc  # noqa: E402
import concourse.mybir as mybir  # noqa: E402
from concourse.bass import AP  # noqa: E402
from concourse.bass_utils import run_bass_kernel_spmd  # noqa: E402
from concourse.tile import TileContext  # noqa: E402

B, C, H, W = 8, 64, 128, 256
D = 64
NW = 576  # per-(c,h) SBUF input row: [L 256 | Rrev 256 | Z 64]
NHOST = 512  # host only ships [L | Rrev]
NH = 16  # h rows per compute group
NG = H // NH  # 8 groups
SBH = 32  # h rows per band-extraction block (and per scratch tensor)
NSB = H // SBH  # 4 band blocks
HSLOT = 32768  # scratch elems per h row
F32 = mybir.dt.float32
F16 = mybir.dt.float16

# input-load blocks: small first block for fast pipeline ramp, spread
# over the three DMA queues
LOAD_BLOCKS = [(0, 16), (16, 16), (32, 32), (64, 32), (96, 32)]

_CACHE = {}


def build():
    nc = bacc.Bacc()
    lr_dram = nc.dram_tensor("lr", [C, H, NHOST], F16, kind="ExternalInput")
    out_dram = nc.dram_tensor("out", [H, W, D], F16, kind="ExternalOutput")
    scr = [
        nc.dram_tensor(f"scratch{k}", [SBH * HSLOT], F16, kind="Internal")
        for k in range(NSB)
    ]

    with TileContext(nc) as tc:
        with (
            tc.tile_pool(name="inp", bufs=1) as pin,
            tc.tile_pool(name="gband", bufs=3) as pg,
            tc.tile_pool(name="ps", bufs=6, space="PSUM") as pps,
        ):
            # whole input resident in SBUF
            lrt = {}  # h -> (tile view, local offset)
            engs = (nc.gpsimd, nc.sync, nc.scalar)
            for i, (hb, nh) in enumerate(LOAD_BLOCKS):
                t = pin.tile([C, nh * NW], F16, tag=f"lr{i}")
                tv = t.rearrange("p (h x) -> p h x", h=nh)
                eng = engs[i % 3]
                eng.dma_start(
                    out=tv[:, :, 0:NHOST],
                    in_=lr_dram[:, hb : hb + nh, :],
                )
                nc.gpsimd.memset(tv[:, :, NHOST:NW], 0.0)
                for hh in range(nh):
                    lrt[hb + hh] = (tv, hh)

            ncopy = 0
            nband = 0
            for g in range(NG):
                h0 = g * NH
                sb = h0 // SBH
                lh0 = h0 - sb * SBH  # local h base within band block

                # stage 2: matmul pairs + PSUM->SBUF casts (DVE 5/8, ACT 3/8)
                g8 = pg.tile([128, NH * 2 * 128], F16, tag="g8")
                g8v = g8.rearrange("p (h jj m) -> p h jj m", h=NH, jj=2)
                for hh in range(NH):
                    lrv, lhh = lrt[h0 + hh]
                    for jj in range(2):
                        pt = pps.tile([128, 128], F32, tag="pt")
                        for par in range(2):
                            j = 2 * jj + par
                            nc.tensor.matmul(
                                pt[par * 64 : par * 64 + 64, :],
                                lhsT=lrv[:, lhh, 64 * j : 64 * j + 64],
                                rhs=lrv[:, lhh, 448 - 64 * j : 576 - 64 * j],
                            )
                        if ncopy % 8 < 5:
                            nc.vector.tensor_copy(g8v[:, hh, jj, :], pt)
                        else:
                            nc.scalar.copy(g8v[:, hh, jj, :], pt)
                        ncopy += 1

                # stage 3: scratch writes, 512B runs, 0.5MB each
                for par in range(2):
                    eng = nc.sync if par == 0 else nc.scalar
                    eng.dma_start(
                        out=AP(
                            scr[sb],
                            lh0 * HSLOT + par * 16384,
                            [[256, 64], [HSLOT, NH], [1, 256]],
                        ),
                        in_=g8[par * 64 : par * 64 + 64, :].rearrange(
                            "p (h x) -> p h x", h=NH
                        ),
                    )

                # stage 4: band extraction for a finished 32-h block,
                # DRAM->DRAM straight into out[h, w, d]
                if lh0 + NH == SBH:
                    hb0 = sb * SBH
                    for par in range(2):
                        for jj in range(2):
                            eng = (nc.sync, nc.scalar, nc.gpsimd)[nband % 3]
                            nband += 1
                            eng.dma_start(
                                out=AP(
                                    out_dram,
                                    hb0 * W * D + jj * 8192 + par * 4096,
                                    [[W * D, SBH], [D, 64], [1, D]],
                                ),
                                in_=AP(
                                    scr[sb],
                                    par * 16384 + jj * 128 + 63,
                                    [[HSLOT, SBH], [255, 64], [1, D]],
                                ),
                            )
    nc.finalize()
    return nc


def kernel(left_feature, right_feature, max_disp):
    assert int(max_disp) == D
    left = np.asarray(left_feature, dtype=np.float32)
    right = np.asarray(right_feature, dtype=np.float32)
    assert left.shape == (B, C, H, W) and right.shape == (B, C, H, W)

    if "nc" not in _CACHE:
        _CACHE["nc"] = build()
    nc = _CACHE["nc"]

    in_maps = []
    for b in range(B):
        lr = np.empty((C, H, NHOST), dtype=np.float16)
        lr[:, :, 0:W] = left[b] / 8
        lr[:, :, W : 2 * W] = right[b, :, :, ::-1] / 8
        in_maps.append({"lr": lr})
    res = run_bass_kernel_spmd(nc, in_maps, list(range(B)))
    _CACHE["last_results"] = res
    out = np.stack(
        [res.results[b]["out"].transpose(2, 0, 1) for b in range(B)], axis=0
    )
    return out.astype(np.float32)


# revision 11
# speedup vs baseline: 1.4338x; 1.1053x over previous
"""Stereo correlation cost volume kernel for Trainium2 (8 NeuronCores).

  out[b, d, h, w] = mean_c( L[b,c,h,w] * R[b,c,h,w-d] )  for w >= d, else 0
  B=8, C=64, H=128, W=256, D=64.

Sharding: data-parallel over batch; core b handles batch b.

Per-core algorithm (w-major blocking, no PE transposes):
  The host ships, per (c, h) row: [L/8 (256) | R reversed /8 (256)], f16;
  64 zero columns per row are memset on-device.  For each h and each
  64-wide w-block j, one matmul
    GT_j[t, m] = sum_c L[c, 64j+t] * R[c, 64j+63-m]        (PSUM fp32)
  directly produces the *transposed* Gram restricted to the 128-wide
  u-window the band needs (u = 64j+63-m; the reversed-R layout makes the
  window a contiguous rhs slice, and for j=0 the u<0 columns read the
  zero pad, so the w<d triangle comes out exactly 0).  Two adjacent
  blocks share one PSUM tile (partitions 0-63 / 64-127 via matmul
  tile_position), so the PSUM->SBUF cast copies run at full 128-lane
  width.  Tiles go to a DRAM scratch with 512-byte-contiguous runs:
    scr[h, par, t, jj, m] at h*32768 + par*16384 + t*256 + jj*128 + m
  (j = 2*jj + par).  The band element out[h, w=64j+t, d] = GT_j[t, 63-t+d]
  then sits at an affine address (... + t*255 + d), so a single
  DRAM->DRAM DMA per (32-h block, par, jj) scatters the band straight
  into the f16 output laid out as [h, w, d].  DMAs are batched large
  (whole input resident in SBUF; 0.5MB scratch writes) and spread over
  all three trigger queues (sync/SP + scalar/ACT HWDGE, gpsimd SWDGE).
  The host transposes to [d, h, w] and casts to fp32.
"""

import os
import sys
import types

import numpy as np

sys.path.insert(0, "/opt/trn_rl_repo")


def _register_axon_hooks():
    # Optional: lets run_bass_kernel_spmd(trace=True) collect NTFF
    # profiles under axon when the image's antenv lacks axon_hooks.
    try:
        import antenv.axon_hooks  # noqa: F401
        return
    except ImportError:
        pass
    try:
        import antenv
        from trn_agent_boot.trn_boot import _ntff_profile_via_ctypes

        m = types.ModuleType("antenv.axon_hooks")
        _hook = _ntff_profile_via_ctypes("/opt/axon/libaxon_pjrt.so")
        m.get_axon_ntff_profile_hook = lambda: _hook
        m.set_axon_ntff_profile_hook = lambda h: None
        sys.modules["antenv.axon_hooks"] = m
        antenv.axon_hooks = m
    except Exception:
        pass


_register_axon_hooks()

import ml_dtypes  # noqa: E402, F401

import concourse.bass as bass  # noqa: E402, F401
import concourse.bacc as bacc  # noqa: E402
import concourse.mybir as mybir  # noqa: E402
from concourse.bass import AP  # noqa: E402
from concourse.bass_utils import run_bass_kernel_spmd  # noqa: E402
from concourse.tile import TileContext  # noqa: E402

B, C, H, W = 8, 64, 128, 256
D = 64
NW = 576  # per-(c,h) SBUF input row: [L 256 | Rrev 256 | Z 64]
NHOST = 512  # host only ships [L | Rrev]
NH = 16  # h rows per compute group
NG = H // NH  # 8 groups
SBH = 16  # h rows per band-extraction block (and per scratch tensor)
NSB = H // SBH  # 8 band blocks
HSLOT = 32768  # scratch elems per h row
F32 = mybir.dt.float32
F16 = mybir.dt.float16

# input-load blocks: small first blocks for fast pipeline ramp, spread
# over the three DMA queues
LOAD_BLOCKS = [(0, 16), (16, 16), (32, 32), (64, 32), (96, 32)]

_CACHE = {}


def build():
    nc = bacc.Bacc()
    lr_dram = nc.dram_tensor("lr", [C, H, NHOST], F16, kind="ExternalInput")
    out_dram = nc.dram_tensor("out", [H, W, D], F16, kind="ExternalOutput")
    scr = [
        nc.dram_tensor(f"scratch{k}", [SBH * HSLOT], F16, kind="Internal")
        for k in range(NSB)
    ]

    with TileContext(nc) as tc:
        with (
            tc.tile_pool(name="inp", bufs=1) as pin,
            tc.tile_pool(name="gband", bufs=3) as pg,
            tc.tile_pool(name="ps", bufs=6, space="PSUM") as pps,
        ):
            # whole input resident in SBUF
            lrt = {}  # h -> (tile view, local offset)
            engs = (nc.sync, nc.scalar, nc.gpsimd, nc.gpsimd, nc.gpsimd)
            for i, (hb, nh) in enumerate(LOAD_BLOCKS):
                t = pin.tile([C, nh * NW], F16, tag=f"lr{i}")
                tv = t.rearrange("p (h x) -> p h x", h=nh)
                eng = engs[i]
                eng.dma_start(
                    out=tv[:, :, 0:NHOST],
                    in_=lr_dram[:, hb : hb + nh, :],
                )
                nc.gpsimd.memset(tv[:, :, NHOST:NW], 0.0)
                for hh in range(nh):
                    lrt[hb + hh] = (tv, hh)

            ncopy = 0
            nband = 0
            for g in range(NG):
                h0 = g * NH
                sb = h0 // SBH
                lh0 = h0 - sb * SBH  # local h base within band block

                # stage 2: matmul pairs + PSUM->SBUF casts (DVE 5/8, ACT 3/8)
                g8 = pg.tile([128, NH * 2 * 128], F16, tag="g8")
                g8v = g8.rearrange("p (h jj m) -> p h jj m", h=NH, jj=2)
                for hh in range(NH):
                    lrv, lhh = lrt[h0 + hh]
                    for jj in range(2):
                        pt = pps.tile([128, 128], F32, tag="pt")
                        for par in range(2):
                            j = 2 * jj + par
                            nc.tensor.matmul(
                                pt[par * 64 : par * 64 + 64, :],
                                lhsT=lrv[:, lhh, 64 * j : 64 * j + 64],
                                rhs=lrv[:, lhh, 448 - 64 * j : 576 - 64 * j],
                            )
                        if ncopy % 8 < 5:
                            nc.vector.tensor_copy(g8v[:, hh, jj, :], pt)
                        else:
                            nc.scalar.copy(g8v[:, hh, jj, :], pt)
                        ncopy += 1

                # stage 3: scratch writes, 512B runs, 0.5MB each
                for par in range(2):
                    eng = nc.sync if par == 0 else nc.scalar
                    eng.dma_start(
                        out=AP(
                            scr[sb],
                            lh0 * HSLOT + par * 16384,
                            [[256, 64], [HSLOT, NH], [1, 256]],
                        ),
                        in_=g8[par * 64 : par * 64 + 64, :].rearrange(
                            "p (h x) -> p h x", h=NH
                        ),
                    )

                # stage 4: band extraction for a finished 32-h block,
                # DRAM->DRAM straight into out[h, w, d]
                if lh0 + NH == SBH:
                    hb0 = sb * SBH
                    for par in range(2):
                        for jj in range(2):
                            eng = nc.gpsimd
                            nband += 1
                            eng.dma_start(
                                out=AP(
                                    out_dram,
                                    hb0 * W * D + jj * 8192 + par * 4096,
                                    [[W * D, SBH], [D, 64], [1, D]],
                                ),
                                in_=AP(
                                    scr[sb],
                                    par * 16384 + jj * 128 + 63,
                                    [[HSLOT, SBH], [255, 64], [1, D]],
                                ),
                            )
    nc.finalize()
    return nc


def kernel(left_feature, right_feature, max_disp):
    assert int(max_disp) == D
    left = np.asarray(left_feature, dtype=np.float32)
    right = np.asarray(right_feature, dtype=np.float32)
    assert left.shape == (B, C, H, W) and right.shape == (B, C, H, W)

    if "nc" not in _CACHE:
        _CACHE["nc"] = build()
    nc = _CACHE["nc"]

    in_maps = []
    for b in range(B):
        lr = np.empty((C, H, NHOST), dtype=np.float16)
        lr[:, :, 0:W] = left[b] / 8
        lr[:, :, W : 2 * W] = right[b, :, :, ::-1] / 8
        in_maps.append({"lr": lr})
    res = run_bass_kernel_spmd(nc, in_maps, list(range(B)))
    _CACHE["last_results"] = res
    out = np.stack(
        [res.results[b]["out"].transpose(2, 0, 1) for b in range(B)], axis=0
    )
    return out.astype(np.float32)


# revision 13
# speedup vs baseline: 1.4770x; 1.0301x over previous
"""Stereo correlation cost volume kernel for Trainium2 (8 NeuronCores).

  out[b, d, h, w] = mean_c( L[b,c,h,w] * R[b,c,h,w-d] )  for w >= d, else 0
  B=8, C=64, H=128, W=256, D=64.

Sharding: data-parallel over batch; core b handles batch b.

Per-core algorithm (w-major blocking, no PE transposes):
  The host ships, per (c, h) row: [L/8 (256) | R reversed /8 (256)], f16;
  64 zero columns per row are memset on-device.  For each h and each
  64-wide w-block j, one matmul
    GT_j[t, m] = sum_c L[c, 64j+t] * R[c, 64j+63-m]        (PSUM fp32)
  directly produces the *transposed* Gram restricted to the 128-wide
  u-window the band needs (u = 64j+63-m; the reversed-R layout makes the
  window a contiguous rhs slice, and for j=0 the u<0 columns read the
  zero pad, so the w<d triangle comes out exactly 0).  Two adjacent
  blocks share one PSUM tile (partitions 0-63 / 64-127 via matmul
  tile_position), so the PSUM->SBUF cast copies run at full 128-lane
  width.  Tiles go to a DRAM scratch with 512-byte-contiguous runs:
    scr[h, par, t, jj, m] at h*32768 + par*16384 + t*256 + jj*128 + m
  (j = 2*jj + par).  The band element out[h, w=64j+t, d] = GT_j[t, 63-t+d]
  then sits at an affine address (... + t*255 + d), so a single
  DRAM->DRAM DMA per (32-h block, par, jj) scatters the band straight
  into the f16 output laid out as [h, w, d].  DMAs are batched large
  (whole input resident in SBUF; 0.5MB scratch writes) and spread over
  all three trigger queues (sync/SP + scalar/ACT HWDGE, gpsimd SWDGE).
  The host transposes to [d, h, w] and casts to fp32.
"""

import os
import sys
import types

import numpy as np

sys.path.insert(0, "/opt/trn_rl_repo")


def _register_axon_hooks():
    # Optional: lets run_bass_kernel_spmd(trace=True) collect NTFF
    # profiles under axon when the image's antenv lacks axon_hooks.
    try:
        import antenv.axon_hooks  # noqa: F401
        return
    except ImportError:
        pass
    try:
        import antenv
        from trn_agent_boot.trn_boot import _ntff_profile_via_ctypes

        m = types.ModuleType("antenv.axon_hooks")
        _hook = _ntff_profile_via_ctypes("/opt/axon/libaxon_pjrt.so")
        m.get_axon_ntff_profile_hook = lambda: _hook
        m.set_axon_ntff_profile_hook = lambda h: None
        sys.modules["antenv.axon_hooks"] = m
        antenv.axon_hooks = m
    except Exception:
        pass


_register_axon_hooks()

import ml_dtypes  # noqa: E402, F401

import concourse.bass as bass  # noqa: E402, F401
import concourse.bacc as bacc  # noqa: E402
import concourse.mybir as mybir  # noqa: E402
from concourse.bass import AP  # noqa: E402
from concourse.bass_utils import run_bass_kernel_spmd  # noqa: E402
from concourse.tile import TileContext  # noqa: E402

B, C, H, W = 8, 64, 128, 256
D = 64
NW = 576  # per-(c,h) SBUF input row: [L 256 | Rrev 256 | Z 64]
NHOST = 512  # host only ships [L | Rrev]
NH = 16  # h rows per compute group
NG = H // NH  # 8 groups
SBH = 16  # h rows per band-extraction block (and per scratch tensor)
NSB = H // SBH  # 8 band blocks
HSLOT = 32768  # scratch elems per h row
F32 = mybir.dt.float32
F16 = mybir.dt.float16

# input-load blocks: small first blocks for fast pipeline ramp, spread
# over the three DMA queues
LOAD_BLOCKS = [(0, 8), (8, 8), (16, 16), (32, 16), (48, 16), (64, 32), (96, 32)]

_CACHE = {}


def build():
    nc = bacc.Bacc()
    lr_dram = nc.dram_tensor("lr", [C, H, NHOST], F16, kind="ExternalInput")
    out_dram = nc.dram_tensor("out", [H, W, D], F16, kind="ExternalOutput")
    scr = [
        nc.dram_tensor(f"scratch{k}", [SBH * HSLOT], F16, kind="Internal")
        for k in range(NSB)
    ]

    with TileContext(nc) as tc:
        with (
            tc.tile_pool(name="inp", bufs=1) as pin,
            tc.tile_pool(name="gband", bufs=3) as pg,
            tc.tile_pool(name="ps", bufs=6, space="PSUM") as pps,
        ):
            # whole input resident in SBUF
            lrt = {}  # h -> (tile view, local offset)
            for i, (hb, nh) in enumerate(LOAD_BLOCKS):
                t = pin.tile([C, nh * NW], F16, tag=f"lr{i}")
                tv = t.rearrange("p (h x) -> p h x", h=nh)
                eng = nc.sync if i % 2 == 0 else nc.scalar
                eng.dma_start(
                    out=tv[:, :, 0:NHOST],
                    in_=lr_dram[:, hb : hb + nh, :],
                )
                nc.gpsimd.memset(tv[:, :, NHOST:NW], 0.0)
                for hh in range(nh):
                    lrt[hb + hh] = (tv, hh)

            ncopy = 0
            nband = 0
            for g in range(NG):
                h0 = g * NH
                sb = h0 // SBH
                lh0 = h0 - sb * SBH  # local h base within band block

                # stage 2: matmul pairs + PSUM->SBUF casts (DVE 5/8, ACT 3/8)
                g8 = pg.tile([128, NH * 2 * 128], F16, tag="g8")
                g8v = g8.rearrange("p (h jj m) -> p h jj m", h=NH, jj=2)
                for hh in range(NH):
                    lrv, lhh = lrt[h0 + hh]
                    for jj in range(2):
                        pt = pps.tile([128, 128], F32, tag="pt")
                        for par in range(2):
                            j = 2 * jj + par
                            nc.tensor.matmul(
                                pt[par * 64 : par * 64 + 64, :],
                                lhsT=lrv[:, lhh, 64 * j : 64 * j + 64],
                                rhs=lrv[:, lhh, 448 - 64 * j : 576 - 64 * j],
                            )
                        if ncopy % 8 < 5:
                            nc.vector.tensor_copy(g8v[:, hh, jj, :], pt)
                        else:
                            nc.scalar.copy(g8v[:, hh, jj, :], pt)
                        ncopy += 1

                # stage 3: scratch writes, 512B runs, 0.5MB each
                for par in range(2):
                    eng = nc.sync if par == 0 else nc.scalar
                    eng.dma_start(
                        out=AP(
                            scr[sb],
                            lh0 * HSLOT + par * 16384,
                            [[256, 64], [HSLOT, NH], [1, 256]],
                        ),
                        in_=g8[par * 64 : par * 64 + 64, :].rearrange(
                            "p (h x) -> p h x", h=NH
                        ),
                    )

                # stage 4: band extraction for a finished 32-h block,
                # DRAM->DRAM straight into out[h, w, d]
                if lh0 + NH == SBH:
                    hb0 = sb * SBH
                    for par in range(2):
                        for jj in range(2):
                            eng = nc.gpsimd
                            nband += 1
                            eng.dma_start(
                                out=AP(
                                    out_dram,
                                    hb0 * W * D + jj * 8192 + par * 4096,
                                    [[W * D, SBH], [D, 64], [1, D]],
                                ),
                                in_=AP(
                                    scr[sb],
                                    par * 16384 + jj * 128 + 63,
                                    [[HSLOT, SBH], [255, 64], [1, D]],
                                ),
                            )
    nc.finalize()
    return nc


def kernel(left_feature, right_feature, max_disp):
    assert int(max_disp) == D
    left = np.asarray(left_feature, dtype=np.float32)
    right = np.asarray(right_feature, dtype=np.float32)
    assert left.shape == (B, C, H, W) and right.shape == (B, C, H, W)

    if "nc" not in _CACHE:
        _CACHE["nc"] = build()
    nc = _CACHE["nc"]

    in_maps = []
    for b in range(B):
        lr = np.empty((C, H, NHOST), dtype=np.float16)
        lr[:, :, 0:W] = left[b] / 8
        lr[:, :, W : 2 * W] = right[b, :, :, ::-1] / 8
        in_maps.append({"lr": lr})
    res = run_bass_kernel_spmd(nc, in_maps, list(range(B)))
    _CACHE["last_results"] = res
    out = np.stack(
        [res.results[b]["out"].transpose(2, 0, 1) for b in range(B)], axis=0
    )
    return out.astype(np.float32)
